# revision 6
# baseline (speedup 1.0000x reference)
"""SupCon loss (nn_CustomLoss_28930899706387) on 8 TRN2 NeuronCores.

Math (per sequence pair b, faithfully mirroring the torch/jax reference):
    cf      = [e0[j]; e1[i]]            # [2P, D], P=1024, D=256
    S       = cf @ cf.T / TEMP          # [2P, 2P]
    m_r     = max_c S[r, c]             # row max (incl. diagonal)
    denom_r = sum_{c != r} exp(S[r,c] - m_r)
    v_r     = (S[r, partner(r)] - m_r) - log(denom_r) + 0 * log(denom_r)
    loss_b  = -(TEMP/BASE_TEMP) * mean_r v_r
    out     = sum_b loss_b

The `+ 0 * log(denom)` term replicates the reference's `pos_mask * log_prob`
elementwise product: when denom == 0 (exp fully underflows), log(denom) = -inf
and 0 * (-inf) = NaN, exactly as the reference's masked sum produces.

Sharding: data-parallel over the batch dim B=8, one pair per NeuronCore.
Each core computes its pair's scalar loss; the host sums the 8 scalars.

Device pipeline per 128-row M-tile of the [2048, 2048] Gram matrix:
    PE : 8 bf16 matmuls (K=256 as 2x128, N=2048 as 4x512) -> PSUM [128, 2048]
    DVE: row max; diag/partner extraction via identity-mask mul+reduce
    ACT: fused exp((G - max)/TEMP) with row-sum accumulation; Ln; small ops
"""

import functools

import numpy as np
import ml_dtypes

import concourse.bass as bass  # noqa: F401  (bass types used via tile/bacc)
import concourse.tile as tile
import concourse.mybir as mybir
from concourse import bacc
from concourse.bass_utils import run_bass_kernel_spmd

B = 8
L = 1024          # positives per pair (P)
D = 256           # embedding dim
P2 = 2 * L        # 2048 = rows of the Gram matrix
NTILE = P2 // 128  # 16 M-tiles
TEMP = 0.07
SCALE = 1.0 / TEMP
N_CORES = 8

F32 = mybir.dt.float32
BF16 = mybir.dt.bfloat16
AF = mybir.ActivationFunctionType
ALU = mybir.AluOpType
AX = mybir.AxisListType


def _build(reps=1):
    """Build the SPMD program. reps>1 repeats the whole compute body (into the
    same accumulators) for steady-state HW timing via wall-clock deltas."""
    nc = bacc.Bacc("TRN2", debug=False, num_devices=N_CORES)
    x = nc.dram_tensor("x", [2 * 128, P2], BF16, kind="ExternalInput")
    ident = nc.dram_tensor("ident", [128, 128], F32, kind="ExternalInput")
    ones = nc.dram_tensor("ones", [128, 1], F32, kind="ExternalInput")
    loss = nc.dram_tensor("loss", [1, 1], F32, kind="ExternalOutput")

    with tile.TileContext(nc) as tc:
        with tc.tile_pool(name="consts", bufs=1) as consts, \
             tc.tile_pool(name="ep", bufs=2) as ep, \
             tc.tile_pool(name="small", bufs=3) as small, \
             tc.tile_pool(name="gp", bufs=2, space="PSUM") as gp:
            xt0 = consts.tile([128, P2], BF16)
            xt1 = consts.tile([128, P2], BF16)
            identt = consts.tile([128, 128], F32)
            onest = consts.tile([128, 1], F32)
            acc = consts.tile([128, NTILE], F32)

            nc.sync.dma_start(identt[:], ident[:, :])
            nc.sync.dma_start(onest[:], ones[:, :])
            # chunked input DMAs so early matmuls can start before the whole
            # 1 MB of X has landed
            for n in range(4):
                cs = slice(512 * n, 512 * (n + 1))
                nc.sync.dma_start(xt0[:, cs], x[0:128, cs])
                nc.sync.dma_start(xt1[:, cs], x[128:256, cs])

            for m in range(NTILE * reps):
                m = m % NTILE
                mc = slice(128 * m, 128 * (m + 1))
                # partner block: rows r and r +/- L are positives of each other
                pc = 128 * m + L if m < NTILE // 2 else 128 * m - L

                g = gp.tile([128, P2], F32, tag="g")
                for n in range(4):
                    ncs = slice(512 * n, 512 * (n + 1))
                    nc.tensor.matmul(g[:, ncs], xt0[:, mc], xt0[:, ncs],
                                     start=True, stop=False)
                    nc.tensor.matmul(g[:, ncs], xt1[:, mc], xt1[:, ncs],
                                     start=False, stop=True)

                mx = small.tile([128, 1], F32, tag="mx")
                nc.vector.reduce_max(mx[:], g[:], axis=AX.X)
                negmx = small.tile([128, 1], F32, tag="negmx")
                nc.scalar.mul(negmx[:], mx[:], -SCALE)

                gpos = small.tile([128, 1], F32, tag="gpos")
                tmpp = small.tile([128, 128], F32, tag="tmpp")
                nc.vector.tensor_mul(tmpp[:], g[:, pc:pc + 128], identt[:])
                nc.vector.reduce_sum(gpos[:], tmpp[:], axis=AX.X)

                # mask the diagonal out of the softmax denominator: push
                # G[r, r] to -3e38 so (G*SCALE + negmx) overflows to -inf and
                # exp contributes exactly 0 — the masked row-sum needs no
                # cancellation-prone `total - exp(diag)` correction.
                # (row max was taken BEFORE this, as the reference does.)
                nc.vector.scalar_tensor_tensor(
                    out=g[:, mc], in0=identt[:], scalar=-3e38, in1=g[:, mc],
                    op0=ALU.mult, op1=ALU.add)

                e = ep.tile([128, P2], F32, tag="e")
                denom = small.tile([128, 1], F32, tag="denom")
                nc.scalar.activation(e[:], g[:], AF.Exp,
                                     bias=negmx[:], scale=SCALE,
                                     accum_out=denom[:])
                lden = small.tile([128, 1], F32, tag="lden")
                nc.scalar.activation(lden[:], denom[:], AF.Ln)
                zl = small.tile([128, 1], F32, tag="zl")
                nc.scalar.mul(zl[:], lden[:], 0.0)
                av = small.tile([128, 1], F32, tag="av")
                nc.scalar.activation(av[:], gpos[:], AF.Identity,
                                     bias=negmx[:], scale=SCALE)
                # acc[:, m] = (av - lden) + zl    (zl = 0, or NaN when denom=0)
                nc.vector.scalar_tensor_tensor(
                    out=acc[:, m:m + 1], in0=av[:], scalar=lden[:], in1=zl[:],
                    op0=ALU.subtract, op1=ALU.add)

            rowtot = small.tile([128, 1], F32, tag="rowtot")
            nc.vector.reduce_sum(rowtot[:], acc[:], axis=AX.X)
            ps = gp.tile([1, 1], F32, tag="g")
            nc.tensor.matmul(ps[:], rowtot[:], onest[:], start=True, stop=True)
            lt = small.tile([1, 1], F32, tag="lt")
            nc.scalar.mul(lt[:], ps[:], -1.0 / P2)
            nc.sync.dma_start(loss[0:1, 0:1], lt[:])

    nc.compile()
    return nc


@functools.lru_cache(maxsize=4)
def _built(reps=1):
    return _build(reps)


def _positive_pairs(l0, l1):
    """Replicate jnp.nonzero(l1[:,None] == l0[None,:], size=P, fill_value=0)."""
    eq = l1[:, None] == l0[None, :]
    i, j = np.nonzero(eq)
    if len(i) >= L:
        i, j = i[:L], j[:L]
    else:
        pad = L - len(i)
        i = np.concatenate([i, np.zeros(pad, dtype=i.dtype)])
        j = np.concatenate([j, np.zeros(pad, dtype=j.dtype)])
    return i, j


def _in_maps(embeddings, labelvecs):
    emb = np.ascontiguousarray(np.asarray(embeddings, dtype=np.float32))
    lv = np.asarray(labelvecs)
    ident = np.eye(128, dtype=np.float32)
    ones = np.ones((128, 1), dtype=np.float32)
    arange = np.arange(L)
    maps = []
    for b in range(B):
        l0, l1 = lv[b], lv[B + b]
        if np.array_equal(l0, arange) and np.array_equal(l1, arange):
            e0c, e1c = emb[b, 0], emb[b, 1]          # identity permutation
        else:
            i, j = _positive_pairs(l0, l1)
            e0c, e1c = emb[b, 0][:, j], emb[b, 1][:, i]
        xb = np.concatenate([e0c, e1c], axis=1).astype(ml_dtypes.bfloat16)
        maps.append({"x": xb, "ident": ident, "ones": ones})
    return maps


def run(embeddings, labelvecs, trace=False, reps=1):
    nc = _built(reps)
    res = run_bass_kernel_spmd(
        nc, _in_maps(embeddings, labelvecs),
        core_ids=list(range(N_CORES)), trace=trace)
    losses = np.stack([r["loss"][0, 0] for r in res.results])
    return np.float32(np.sum(losses)), res


def kernel(embeddings, embeddings_mask, labelvecs):
    del embeddings_mask  # all-False by construction: masked select is identity
    out, _ = run(embeddings, labelvecs)
    return np.asarray(out, dtype=np.float32)
